# revision 1
# baseline (speedup 1.0000x reference)
"""GCNConv mean-aggregation kernel for 8 Trainium2 NeuronCores.

Reference computation:
    msgs   = x[src]                       # [E, D] gather
    summed = segment_sum(msgs, dst, N)    # [N, D]
    deg    = segment_sum(ones, dst, N)    # [N]
    h      = summed / max(deg, 1)
    out    = h @ W.T + b                  # [N, D_OUT]

Strategy (no collectives needed):
  - Shard edges by contiguous dst ranges: core c owns nodes
    [c*6272, (c+1)*6272).  Each core fully reduces its own node range.
  - Per core the edge stream is grouped into 64-node dst windows.  For
    each 128-edge subtile we gather x[src] rows from HBM with big
    dma_gather calls (512B rows: 64 feats + 1.0 weight col + pad)
    rotated over the 4 SWDGE queues (single_packet=False; the default
    single-packet mode wedges the SDMA engine beyond ~64 descs/lane),
    build a [128e, 64n] one-hot from dst via a DVE is_equal against an
    iota, and accumulate  onehot.T @ msgs  into a [64, 65] PSUM tile
    (features + degree in one matmul chain).
  - Normalize by max(deg,1) with per-partition scalars, transpose h via
    the PE identity trick, apply W (as lhsT = W.T) and bias, and write
    out.T slices ([64, 6272] per core).  Host reassembles/transposes.
  - dma_gather indices are int16, so x is staged into two gather tables
    (src < 32767 and src >= 32767), each with a zero row at index 0
    used by padding edges (contributes 0 to sums and degree).
"""

import sys

sys.path.insert(0, "/opt/trn_rl_repo")

import numpy as np

import concourse.bacc as bacc
import concourse.mybir as mybir
import concourse.tile as tile
from concourse.bass_utils import run_bass_kernel_spmd

N_NODES = 50000
N_EDGES = 800000
D = 64
N_CORES = 8
NPC = 6272          # nodes per core (= 98 windows of 64 = 49 tiles of 128)
WIN = 64            # dst-window width per PSUM accumulation group
N_WIN = NPC // WIN  # 98
SPLIT = 32767       # src < SPLIT -> lo table, else hi table
ROW = 128           # gather row: 64 feats + weight + zero pad (512 B)
CHUNK = 16          # subtiles (of 128 edges) per dma_gather call
NQ = 4              # SWDGE queues for parallel gather descriptor work

F32 = mybir.dt.float32
I16 = mybir.dt.int16

# Results of the most recent run (for test harness inspection).
LAST = {}


def _prep(x, src, dst):
    """Host-side sharding: build gather tables, per-core padded edge
    streams (int16 gather idx + f32 dst-rel), and per-window subtile
    budgets (shared across cores; SPMD program structure)."""
    x = np.asarray(x, dtype=np.float32)
    src = np.asarray(src, dtype=np.int64)
    dst = np.asarray(dst, dtype=np.int64)

    n_lo = SPLIT
    n_hi = N_NODES - SPLIT
    xlo = np.zeros((n_lo + 1, ROW), dtype=np.float32)
    xlo[1:, :D] = x[:SPLIT]
    xlo[1:, D] = 1.0
    xhi = np.zeros((n_hi + 1, ROW), dtype=np.float32)
    xhi[1:, :D] = x[SPLIT:]
    xhi[1:, D] = 1.0

    gw = (dst // WIN).astype(np.int64)
    cls = (src >= SPLIT).astype(np.int64)
    key = gw * 2 + cls
    order = np.argsort(key, kind="stable")
    src_s, dst_s = src[order], dst[order]

    n_groups = (N_CORES * N_WIN) * 2
    counts = np.bincount(key[order], minlength=n_groups)
    starts = np.zeros(n_groups + 1, dtype=np.int64)
    np.cumsum(counts, out=starts[1:])

    cnt = counts.reshape(N_CORES, N_WIN, 2)
    kA = np.maximum(1, -(-cnt[:, :, 0].max(axis=0) // 128))  # [N_WIN]
    kB = -(-cnt[:, :, 1].max(axis=0) // 128)                  # [N_WIN]
    SA = int(kA.sum())
    SB = int(kB.sum())

    idx_lo = (src_s + 1).astype(np.int16)
    idx_hi = (src_s - SPLIT + 1).astype(np.int16)

    offA = np.zeros(N_WIN + 1, dtype=np.int64)
    np.cumsum(kA, out=offA[1:])
    offB = np.zeros(N_WIN + 1, dtype=np.int64)
    np.cumsum(kB, out=offB[1:])

    per_core = []
    for c in range(N_CORES):
        iA = np.zeros(SA * 128, dtype=np.int16)
        dA = np.zeros(SA * 128, dtype=np.float32)
        iB = np.zeros(max(SB, 1) * 128, dtype=np.int16)
        dB = np.zeros(max(SB, 1) * 128, dtype=np.float32)
        for w in range(N_WIN):
            g = (c * N_WIN + w) * 2
            base = (c * N_WIN + w) * WIN
            s0, s1 = starts[g], starts[g + 1]
            p0 = int(offA[w]) * 128
            iA[p0 : p0 + (s1 - s0)] = idx_lo[s0:s1]
            dA[p0 : p0 + (s1 - s0)] = (dst_s[s0:s1] - base).astype(np.float32)
            s0, s1 = starts[g + 1], starts[g + 2]
            p0 = int(offB[w]) * 128
            iB[p0 : p0 + (s1 - s0)] = idx_hi[s0:s1]
            dB[p0 : p0 + (s1 - s0)] = (dst_s[s0:s1] - base).astype(np.float32)
        per_core.append((iA, dA, iB, dB))

    return xlo, xhi, kA, kB, SA, SB, offA, offB, per_core


def _wrap_idx(idx_flat):
    """int16 stream -> dma_gather layout [128, n/16]: value i at
    [i % 16, i // 16], replicated across the 8 groups of 16 partitions."""
    a = idx_flat.reshape(-1, 16).T
    return np.tile(a, (8, 1)).copy()


def _wrap_dst(d_flat):
    """f32 stream -> [128, S]: subtile s lane e at [e, s]."""
    return np.ascontiguousarray(d_flat.reshape(-1, 128).T)


def _build_program(kA, kB, SA, SB, offA, offB):
    nc = bacc.Bacc(
        "TRN2", target_bir_lowering=False, debug=False, num_swdge_queues=NQ
    )

    t_xlo = nc.dram_tensor("xlo", [SPLIT + 1, ROW], F32, kind="ExternalInput")
    t_xhi = nc.dram_tensor(
        "xhi", [N_NODES - SPLIT + 1, ROW], F32, kind="ExternalInput"
    )
    t_wt = nc.dram_tensor("wt", [D, D], F32, kind="ExternalInput")
    t_b = nc.dram_tensor("bias", [D, 1], F32, kind="ExternalInput")
    t_ia = nc.dram_tensor("idxa", [128, SA * 8], I16, kind="ExternalInput")
    t_da = nc.dram_tensor("dsta", [128, SA], F32, kind="ExternalInput")
    SBp = max(SB, 1)
    t_ib = nc.dram_tensor("idxb", [128, SBp * 8], I16, kind="ExternalInput")
    t_db = nc.dram_tensor("dstb", [128, SBp], F32, kind="ExternalInput")
    t_iota = nc.dram_tensor("iota", [128, CHUNK * WIN], F32, kind="ExternalInput")
    t_id = nc.dram_tensor("ident", [D, D], F32, kind="ExternalInput")
    t_out = nc.dram_tensor("out", [D, NPC], F32, kind="ExternalOutput")

    callsA = [(p, min(CHUNK, SA - p)) for p in range(0, SA, CHUNK)]
    callsB = [(p, min(CHUNK, SB - p)) for p in range(0, SB, CHUNK)]

    with tile.TileContext(nc) as tc:
        with (
            tc.tile_pool(name="const", bufs=1) as cpool,
            tc.tile_pool(name="idx", bufs=1) as ipool,
            tc.tile_pool(name="msgsa", bufs=4) as mpa,
            tc.tile_pool(name="msgsb", bufs=3) as mpb,
            tc.tile_pool(name="oha", bufs=4) as opa,
            tc.tile_pool(name="ohb", bufs=3) as opb,
            tc.tile_pool(name="norm", bufs=4) as npool,
            tc.tile_pool(name="hpo", bufs=2) as hpool,
            tc.tile_pool(name="psacc", bufs=4, space="PSUM") as ps_acc,
            tc.tile_pool(name="pstr", bufs=2, space="PSUM") as ps_tr,
            tc.tile_pool(name="psz", bufs=2, space="PSUM") as ps_z,
        ):
            # ---- constants (iota / identity supplied from host) ----
            ident = cpool.tile([D, D], F32)
            nc.sync.dma_start(out=ident[:], in_=t_id[:])
            wt_sb = cpool.tile([D, D], F32)
            nc.sync.dma_start(out=wt_sb[:], in_=t_wt[:])
            b_sb = cpool.tile([D, 1], F32)
            nc.sync.dma_start(out=b_sb[:], in_=t_b[:])
            iota_f = cpool.tile([128, CHUNK * WIN], F32)
            nc.sync.dma_start(out=iota_f[:], in_=t_iota[:])

            ia_sb = ipool.tile([128, SA * 8], I16)
            nc.sync.dma_start(out=ia_sb[:], in_=t_ia[:])
            da_sb = ipool.tile([128, SA], F32)
            nc.sync.dma_start(out=da_sb[:], in_=t_da[:])
            ib_sb = ipool.tile([128, SBp * 8], I16)
            nc.sync.dma_start(out=ib_sb[:], in_=t_ib[:])
            db_sb = ipool.tile([128, SBp], F32)
            nc.sync.dma_start(out=db_sb[:], in_=t_db[:])

            out_sb = cpool.tile([D, NPC], F32)

            chunk_tiles = {0: [], 1: []}
            call_no = [0]

            def emit_chunk(st, k):
                if st == 0:
                    pos, nsub = callsA[k]
                    mp, op, tsrc, isb, dsb = mpa, opa, t_xlo, ia_sb, da_sb
                else:
                    pos, nsub = callsB[k]
                    mp, op, tsrc, isb, dsb = mpb, opb, t_xhi, ib_sb, db_sb
                msgs = mp.tile([128, CHUNK, ROW], F32)
                nidx = nsub * 128
                # single_packet=False: one packet per descriptor. The default
                # coalesces the whole call into one SDMA packet, which wedges
                # the engine beyond ~64 descriptors/lane (num_idxs >~ 1000).
                # Rotating queue_num spreads descriptor generation + ring
                # drain over the 4 SWDGE queues (~2x measured).
                nc.gpsimd.dma_gather(
                    msgs[:, :nsub, :],
                    tsrc[:],
                    isb[:, pos * 8 : pos * 8 + nsub * 8],
                    nidx,
                    nidx,
                    ROW,
                    single_packet=False,
                    queue_num=call_no[0] % NQ,
                )
                call_no[0] += 1
                oh = op.tile([128, CHUNK * WIN], F32)
                dst_b = (
                    dsb[:, pos : pos + nsub]
                    .unsqueeze(2)
                    .to_broadcast([128, nsub, WIN])
                )
                nc.vector.tensor_tensor(
                    out=oh[:, : nsub * WIN].rearrange("p (s w) -> p s w", w=WIN),
                    in0=iota_f[:, : nsub * WIN].rearrange(
                        "p (s w) -> p s w", w=WIN
                    ),
                    in1=dst_b,
                    op=mybir.AluOpType.is_equal,
                )
                chunk_tiles[st].append((msgs, oh))

            cursor = [0, 0]

            def tiles_for(st, s):
                k = s // CHUNK
                while cursor[st] <= k:
                    emit_chunk(st, cursor[st])
                    cursor[st] += 1
                msgs, oh = chunk_tiles[st][k]
                return msgs, oh, s % CHUNK

            pst = None
            for w in range(N_WIN):
                subs = [(0, int(offA[w]) + j) for j in range(int(kA[w]))]
                subs += [(1, int(offB[w]) + j) for j in range(int(kB[w]))]
                ps = ps_acc.tile([WIN, D + 1], F32)
                for j, (st, s) in enumerate(subs):
                    msgs, oh, col = tiles_for(st, s)
                    nc.tensor.matmul(
                        out=ps[:],
                        lhsT=oh[:, col * WIN : (col + 1) * WIN],
                        rhs=msgs[:, col, : D + 1],
                        start=(j == 0),
                        stop=(j == len(subs) - 1),
                    )
                deg = npool.tile([WIN, 1], F32)
                nc.vector.tensor_scalar_max(deg[:], ps[:, D : D + 1], 1.0)
                rec = npool.tile([WIN, 1], F32)
                nc.vector.reciprocal(rec[:], deg[:])
                h_w = npool.tile([WIN, D], F32)
                nc.vector.tensor_scalar_mul(h_w[:], ps[:, :D], rec[:])
                half = w % 2
                if half == 0:
                    pst = ps_tr.tile([D, 128], F32)
                nc.tensor.transpose(
                    out=pst[:, half * WIN : half * WIN + WIN],
                    in_=h_w[:],
                    identity=ident[:],
                )
                if half == 1:
                    ht = hpool.tile([D, 128], F32)
                    nc.vector.tensor_copy(out=ht[:], in_=pst[:])
                    z = ps_z.tile([D, 128], F32)
                    nc.tensor.matmul(
                        out=z[:], lhsT=wt_sb[:], rhs=ht[:], start=True, stop=True
                    )
                    t0 = (w // 2) * 128
                    nc.vector.tensor_scalar_add(
                        out_sb[:, t0 : t0 + 128], z[:], b_sb[:]
                    )

            nc.sync.dma_start(out=t_out[:], in_=out_sb[:])

    nc.compile()
    return nc


def kernel(x, src, dst, W, b):
    x = np.asarray(x, dtype=np.float32)
    W = np.asarray(W, dtype=np.float32)
    b = np.asarray(b, dtype=np.float32)

    xlo, xhi, kA, kB, SA, SB, offA, offB, per_core = _prep(x, src, dst)
    nc = _build_program(kA, kB, SA, SB, offA, offB)

    wt = np.ascontiguousarray(W.T)
    bcol = np.ascontiguousarray(b.reshape(D, 1))
    iota_arr = np.tile(
        np.arange(WIN, dtype=np.float32)[None, :], (128, CHUNK)
    ).copy()
    ident_arr = np.eye(D, dtype=np.float32)

    in_maps = []
    for c in range(N_CORES):
        iA, dA, iB, dB = per_core[c]
        in_maps.append(
            {
                "xlo": xlo,
                "xhi": xhi,
                "wt": wt,
                "bias": bcol,
                "idxa": _wrap_idx(iA),
                "dsta": _wrap_dst(dA),
                "idxb": _wrap_idx(iB),
                "dstb": _wrap_dst(dB),
                "iota": iota_arr,
                "ident": ident_arr,
            }
        )

    res = run_bass_kernel_spmd(nc, in_maps, list(range(N_CORES)))
    LAST["results"] = res
    LAST["exec_time_ns"] = res.exec_time_ns

    out_t = np.concatenate([res.results[c]["out"] for c in range(N_CORES)], axis=1)
    return np.ascontiguousarray(out_t.T[:N_NODES])



# revision 3
# speedup vs baseline: 1.6774x; 1.6774x over previous
"""GCNConv mean-aggregation kernel for 8 Trainium2 NeuronCores.

Reference computation:
    msgs   = x[src]                       # [E, D] gather
    summed = segment_sum(msgs, dst, N)    # [N, D]
    deg    = segment_sum(ones, dst, N)    # [N]
    h      = summed / max(deg, 1)
    out    = h @ W.T + b                  # [N, D_OUT]

Strategy (no collectives needed):
  - Shard edges by contiguous dst ranges: core c owns nodes
    [c*6272, (c+1)*6272).  Each core fully reduces its own node range.
  - Per core the edge stream is grouped into 64-node dst windows.  For
    each 128-edge subtile we gather x[src] rows from HBM with big
    dma_gather calls (512B rows: 64 feats + 1.0 weight col + pad)
    rotated over the 4 SWDGE queues (single_packet=False; the default
    single-packet mode wedges the SDMA engine beyond ~64 descs/lane),
    build a [128e, 64n] one-hot from dst via a DVE is_equal against an
    iota, and accumulate  onehot.T @ msgs  into a [64, 65] PSUM tile
    (features + degree in one matmul chain).
  - Normalize by max(deg,1) with per-partition scalars, transpose h via
    the PE identity trick, apply W (as lhsT = W.T) and bias, and write
    out.T slices ([64, 6272] per core).  Host reassembles/transposes.
  - dma_gather indices are int16, so x is staged into two gather tables
    (src < 32767 and src >= 32767), each with a zero row at index 0
    used by padding edges (contributes 0 to sums and degree).
"""

import sys

sys.path.insert(0, "/opt/trn_rl_repo")

import numpy as np

import concourse.bacc as bacc
import concourse.mybir as mybir
import concourse.tile as tile
from concourse.bass_utils import run_bass_kernel_spmd

N_NODES = 50000
N_EDGES = 800000
D = 64
N_CORES = 8
NPC = 6272          # nodes per core (= 98 windows of 64 = 49 tiles of 128)
WIN = 64            # dst-window width per PSUM accumulation group
N_WIN = NPC // WIN  # 98
SPLIT = 32767       # src < SPLIT -> lo table, else hi table
ROW = 128           # gather row: 64 feats + weight + zero pad (512 B)
CHUNK = 8           # subtiles (of 128 edges) per dma_gather call
NQ = 4              # SWDGE queues for parallel gather descriptor work

F32 = mybir.dt.float32
I16 = mybir.dt.int16

# Results of the most recent run (for test harness inspection).
LAST = {}


def _prep(x, src, dst):
    """Host-side sharding: build gather tables, per-core padded edge
    streams (int16 gather idx + f32 dst-rel), and per-window subtile
    budgets (shared across cores; SPMD program structure)."""
    x = np.asarray(x, dtype=np.float32)
    src = np.asarray(src, dtype=np.int64)
    dst = np.asarray(dst, dtype=np.int64)

    n_lo = SPLIT
    n_hi = N_NODES - SPLIT
    xlo = np.zeros((n_lo + 1, ROW), dtype=np.float32)
    xlo[1:, :D] = x[:SPLIT]
    xlo[1:, D] = 1.0
    xhi = np.zeros((n_hi + 1, ROW), dtype=np.float32)
    xhi[1:, :D] = x[SPLIT:]
    xhi[1:, D] = 1.0

    gw = (dst // WIN).astype(np.int64)
    cls = (src >= SPLIT).astype(np.int64)
    key = gw * 2 + cls
    order = np.argsort(key, kind="stable")
    src_s, dst_s = src[order], dst[order]

    n_groups = (N_CORES * N_WIN) * 2
    counts = np.bincount(key[order], minlength=n_groups)
    starts = np.zeros(n_groups + 1, dtype=np.int64)
    np.cumsum(counts, out=starts[1:])

    cnt = counts.reshape(N_CORES, N_WIN, 2)
    kA = np.maximum(1, -(-cnt[:, :, 0].max(axis=0) // 128))  # [N_WIN]
    kB = -(-cnt[:, :, 1].max(axis=0) // 128)                  # [N_WIN]
    SA = int(kA.sum())
    SB = int(kB.sum())

    idx_lo = (src_s + 1).astype(np.int16)
    idx_hi = (src_s - SPLIT + 1).astype(np.int16)

    offA = np.zeros(N_WIN + 1, dtype=np.int64)
    np.cumsum(kA, out=offA[1:])
    offB = np.zeros(N_WIN + 1, dtype=np.int64)
    np.cumsum(kB, out=offB[1:])

    per_core = []
    for c in range(N_CORES):
        iA = np.zeros(SA * 128, dtype=np.int16)
        dA = np.zeros(SA * 128, dtype=np.float32)
        iB = np.zeros(max(SB, 1) * 128, dtype=np.int16)
        dB = np.zeros(max(SB, 1) * 128, dtype=np.float32)
        for w in range(N_WIN):
            g = (c * N_WIN + w) * 2
            base = (c * N_WIN + w) * WIN
            s0, s1 = starts[g], starts[g + 1]
            p0 = int(offA[w]) * 128
            iA[p0 : p0 + (s1 - s0)] = idx_lo[s0:s1]
            dA[p0 : p0 + (s1 - s0)] = (dst_s[s0:s1] - base).astype(np.float32)
            s0, s1 = starts[g + 1], starts[g + 2]
            p0 = int(offB[w]) * 128
            iB[p0 : p0 + (s1 - s0)] = idx_hi[s0:s1]
            dB[p0 : p0 + (s1 - s0)] = (dst_s[s0:s1] - base).astype(np.float32)
        per_core.append((iA, dA, iB, dB))

    return xlo, xhi, kA, kB, SA, SB, offA, offB, per_core


def _wrap_idx(idx_flat):
    """int16 stream -> dma_gather layout [128, n/16]: value i at
    [i % 16, i // 16], replicated across the 8 groups of 16 partitions."""
    a = idx_flat.reshape(-1, 16).T
    return np.tile(a, (8, 1)).copy()


def _wrap_dst(d_flat):
    """f32 stream -> [128, S]: subtile s lane e at [e, s]."""
    return np.ascontiguousarray(d_flat.reshape(-1, 128).T)


def _build_program(kA, kB, SA, SB, offA, offB):
    nc = bacc.Bacc(
        "TRN2", target_bir_lowering=False, debug=False, num_swdge_queues=NQ
    )

    t_xlo = nc.dram_tensor("xlo", [SPLIT + 1, ROW], F32, kind="ExternalInput")
    t_xhi = nc.dram_tensor(
        "xhi", [N_NODES - SPLIT + 1, ROW], F32, kind="ExternalInput"
    )
    t_wt = nc.dram_tensor("wt", [D, D], F32, kind="ExternalInput")
    t_b = nc.dram_tensor("bias", [D, 1], F32, kind="ExternalInput")
    t_ia = nc.dram_tensor("idxa", [128, SA * 8], I16, kind="ExternalInput")
    t_da = nc.dram_tensor("dsta", [128, SA], F32, kind="ExternalInput")
    SBp = max(SB, 1)
    t_ib = nc.dram_tensor("idxb", [128, SBp * 8], I16, kind="ExternalInput")
    t_db = nc.dram_tensor("dstb", [128, SBp], F32, kind="ExternalInput")
    t_iota = nc.dram_tensor("iota", [128, CHUNK * WIN], F32, kind="ExternalInput")
    t_id = nc.dram_tensor("ident", [D, D], F32, kind="ExternalInput")
    t_out = nc.dram_tensor("out", [D, NPC], F32, kind="ExternalOutput")

    callsA = [(p, min(CHUNK, SA - p)) for p in range(0, SA, CHUNK)]
    callsB = [(p, min(CHUNK, SB - p)) for p in range(0, SB, CHUNK)]

    with tile.TileContext(nc) as tc:
        with (
            tc.tile_pool(name="const", bufs=1) as cpool,
            tc.tile_pool(name="idx", bufs=1) as ipool,
            tc.tile_pool(name="msgsa", bufs=4) as mpa,
            tc.tile_pool(name="msgsb", bufs=3) as mpb,
            tc.tile_pool(name="oha", bufs=4) as opa,
            tc.tile_pool(name="ohb", bufs=3) as opb,
            tc.tile_pool(name="norm", bufs=4) as npool,
            tc.tile_pool(name="hpo", bufs=2) as hpool,
            tc.tile_pool(name="psacc", bufs=4, space="PSUM") as ps_acc,
            tc.tile_pool(name="pstr", bufs=2, space="PSUM") as ps_tr,
            tc.tile_pool(name="psz", bufs=2, space="PSUM") as ps_z,
        ):
            # ---- constants (iota / identity supplied from host) ----
            ident = cpool.tile([D, D], F32)
            nc.sync.dma_start(out=ident[:], in_=t_id[:])
            wt_sb = cpool.tile([D, D], F32)
            nc.sync.dma_start(out=wt_sb[:], in_=t_wt[:])
            b_sb = cpool.tile([D, 1], F32)
            nc.sync.dma_start(out=b_sb[:], in_=t_b[:])
            iota_f = cpool.tile([128, CHUNK * WIN], F32)
            nc.sync.dma_start(out=iota_f[:], in_=t_iota[:])

            ia_sb = ipool.tile([128, SA * 8], I16)
            nc.sync.dma_start(out=ia_sb[:], in_=t_ia[:])
            da_sb = ipool.tile([128, SA], F32)
            nc.sync.dma_start(out=da_sb[:], in_=t_da[:])
            ib_sb = ipool.tile([128, SBp * 8], I16)
            nc.sync.dma_start(out=ib_sb[:], in_=t_ib[:])
            db_sb = ipool.tile([128, SBp], F32)
            nc.sync.dma_start(out=db_sb[:], in_=t_db[:])

            out_sb = cpool.tile([D, NPC], F32)

            chunk_tiles = {0: [], 1: []}
            call_no = [0]

            def emit_chunk(st, k):
                if st == 0:
                    pos, nsub = callsA[k]
                    mp, op, tsrc, isb, dsb = mpa, opa, t_xlo, ia_sb, da_sb
                else:
                    pos, nsub = callsB[k]
                    mp, op, tsrc, isb, dsb = mpb, opb, t_xhi, ib_sb, db_sb
                msgs = mp.tile([128, CHUNK, ROW], F32)
                nidx = nsub * 128
                # single_packet=False: one packet per descriptor. The default
                # coalesces the whole call into one SDMA packet, which wedges
                # the engine beyond ~64 descriptors/lane (num_idxs >~ 1000).
                # Rotating queue_num spreads descriptor generation + ring
                # drain over the 4 SWDGE queues (~2x measured).
                nc.gpsimd.dma_gather(
                    msgs[:, :nsub, :],
                    tsrc[:],
                    isb[:, pos * 8 : pos * 8 + nsub * 8],
                    nidx,
                    nidx,
                    ROW,
                    single_packet=True,
                    queue_num=call_no[0] % NQ,
                )
                call_no[0] += 1
                oh = op.tile([128, CHUNK * WIN], F32)
                dst_b = (
                    dsb[:, pos : pos + nsub]
                    .unsqueeze(2)
                    .to_broadcast([128, nsub, WIN])
                )
                nc.vector.tensor_tensor(
                    out=oh[:, : nsub * WIN].rearrange("p (s w) -> p s w", w=WIN),
                    in0=iota_f[:, : nsub * WIN].rearrange(
                        "p (s w) -> p s w", w=WIN
                    ),
                    in1=dst_b,
                    op=mybir.AluOpType.is_equal,
                )
                chunk_tiles[st].append((msgs, oh))

            cursor = [0, 0]

            def tiles_for(st, s):
                k = s // CHUNK
                while cursor[st] <= k:
                    emit_chunk(st, cursor[st])
                    cursor[st] += 1
                msgs, oh = chunk_tiles[st][k]
                return msgs, oh, s % CHUNK

            pst = None
            for w in range(N_WIN):
                subs = [(0, int(offA[w]) + j) for j in range(int(kA[w]))]
                subs += [(1, int(offB[w]) + j) for j in range(int(kB[w]))]
                ps = ps_acc.tile([WIN, D + 1], F32)
                for j, (st, s) in enumerate(subs):
                    msgs, oh, col = tiles_for(st, s)
                    nc.tensor.matmul(
                        out=ps[:],
                        lhsT=oh[:, col * WIN : (col + 1) * WIN],
                        rhs=msgs[:, col, : D + 1],
                        start=(j == 0),
                        stop=(j == len(subs) - 1),
                    )
                deg = npool.tile([WIN, 1], F32)
                nc.vector.tensor_scalar_max(deg[:], ps[:, D : D + 1], 1.0)
                rec = npool.tile([WIN, 1], F32)
                nc.vector.reciprocal(rec[:], deg[:])
                h_w = npool.tile([WIN, D], F32)
                nc.vector.tensor_scalar_mul(h_w[:], ps[:, :D], rec[:])
                half = w % 2
                if half == 0:
                    pst = ps_tr.tile([D, 128], F32)
                nc.tensor.transpose(
                    out=pst[:, half * WIN : half * WIN + WIN],
                    in_=h_w[:],
                    identity=ident[:],
                )
                if half == 1:
                    ht = hpool.tile([D, 128], F32)
                    nc.vector.tensor_copy(out=ht[:], in_=pst[:])
                    z = ps_z.tile([D, 128], F32)
                    nc.tensor.matmul(
                        out=z[:], lhsT=wt_sb[:], rhs=ht[:], start=True, stop=True
                    )
                    t0 = (w // 2) * 128
                    nc.vector.tensor_scalar_add(
                        out_sb[:, t0 : t0 + 128], z[:], b_sb[:]
                    )

            nc.sync.dma_start(out=t_out[:], in_=out_sb[:])

    nc.compile()
    return nc


def kernel(x, src, dst, W, b):
    x = np.asarray(x, dtype=np.float32)
    W = np.asarray(W, dtype=np.float32)
    b = np.asarray(b, dtype=np.float32)

    xlo, xhi, kA, kB, SA, SB, offA, offB, per_core = _prep(x, src, dst)
    nc = _build_program(kA, kB, SA, SB, offA, offB)

    wt = np.ascontiguousarray(W.T)
    bcol = np.ascontiguousarray(b.reshape(D, 1))
    iota_arr = np.tile(
        np.arange(WIN, dtype=np.float32)[None, :], (128, CHUNK)
    ).copy()
    ident_arr = np.eye(D, dtype=np.float32)

    in_maps = []
    for c in range(N_CORES):
        iA, dA, iB, dB = per_core[c]
        in_maps.append(
            {
                "xlo": xlo,
                "xhi": xhi,
                "wt": wt,
                "bias": bcol,
                "idxa": _wrap_idx(iA),
                "dsta": _wrap_dst(dA),
                "idxb": _wrap_idx(iB),
                "dstb": _wrap_dst(dB),
                "iota": iota_arr,
                "ident": ident_arr,
            }
        )

    res = run_bass_kernel_spmd(nc, in_maps, list(range(N_CORES)))
    LAST["results"] = res
    LAST["exec_time_ns"] = res.exec_time_ns

    out_t = np.concatenate([res.results[c]["out"] for c in range(N_CORES)], axis=1)
    return np.ascontiguousarray(out_t.T[:N_NODES])



# revision 5
# speedup vs baseline: 1.7500x; 1.0433x over previous
"""GCNConv mean-aggregation kernel for 8 Trainium2 NeuronCores.

Reference computation:
    msgs   = x[src]                       # [E, D] gather
    summed = segment_sum(msgs, dst, N)    # [N, D]
    deg    = segment_sum(ones, dst, N)    # [N]
    h      = summed / max(deg, 1)
    out    = h @ W.T + b                  # [N, D_OUT]

Strategy (no collectives needed):
  - Shard edges by contiguous dst ranges: core c owns nodes
    [c*6272, (c+1)*6272).  Each core fully reduces its own node range.
  - Per core the edge stream is grouped into 64-node dst windows.  For
    each 128-edge subtile we gather x[src] rows from HBM with big
    dma_gather calls (512B rows: 64 feats + 1.0 weight col + pad)
    rotated over the 4 SWDGE queues (single_packet=False; the default
    single-packet mode wedges the SDMA engine beyond ~64 descs/lane),
    build a [128e, 64n] one-hot from dst via a DVE is_equal against an
    iota, and accumulate  onehot.T @ msgs  into a [64, 65] PSUM tile
    (features + degree in one matmul chain).
  - Normalize by max(deg,1) with per-partition scalars, transpose h via
    the PE identity trick, apply W (as lhsT = W.T) and bias, and write
    out.T slices ([64, 6272] per core).  Host reassembles/transposes.
  - dma_gather indices are int16, so x is staged into two gather tables
    (src < 32767 and src >= 32767), each with a zero row at index 0
    used by padding edges (contributes 0 to sums and degree).
"""

import sys

sys.path.insert(0, "/opt/trn_rl_repo")

import numpy as np

import concourse.bacc as bacc
import concourse.mybir as mybir
import concourse.tile as tile
from concourse.bass_utils import run_bass_kernel_spmd

N_NODES = 50000
N_EDGES = 800000
D = 64
N_CORES = 8
NPC = 6272          # nodes per core (= 98 windows of 64 = 49 tiles of 128)
WIN = 64            # dst-window width per PSUM accumulation group
N_WIN = NPC // WIN  # 98
SPLIT = 32767       # src < SPLIT -> lo table, else hi table
ROW = 128           # gather row: 64 feats + weight + zero pad (512 B)
CHUNK = 8           # subtiles (of 128 edges) per dma_gather call
NQ = 4              # SWDGE queues for parallel gather descriptor work

F32 = mybir.dt.float32
I16 = mybir.dt.int16

# Results of the most recent run (for test harness inspection).
LAST = {}


def _prep(x, src, dst):
    """Host-side sharding: build gather tables, per-core padded edge
    streams (int16 gather idx + f32 dst-rel), and per-window subtile
    budgets (shared across cores; SPMD program structure)."""
    x = np.asarray(x, dtype=np.float32)
    src = np.asarray(src, dtype=np.int64)
    dst = np.asarray(dst, dtype=np.int64)

    n_lo = SPLIT
    n_hi = N_NODES - SPLIT
    xlo = np.zeros((n_lo + 1, ROW), dtype=np.float32)
    xlo[1:, :D] = x[:SPLIT]
    xlo[1:, D] = 1.0
    xhi = np.zeros((n_hi + 1, ROW), dtype=np.float32)
    xhi[1:, :D] = x[SPLIT:]
    xhi[1:, D] = 1.0

    gw = (dst // WIN).astype(np.int64)
    cls = (src >= SPLIT).astype(np.int64)
    key = gw * 2 + cls
    order = np.argsort(key, kind="stable")
    src_s, dst_s = src[order], dst[order]

    n_groups = (N_CORES * N_WIN) * 2
    counts = np.bincount(key[order], minlength=n_groups)
    starts = np.zeros(n_groups + 1, dtype=np.int64)
    np.cumsum(counts, out=starts[1:])

    cnt = counts.reshape(N_CORES, N_WIN, 2)
    kA = np.maximum(1, -(-cnt[:, :, 0].max(axis=0) // 128))  # [N_WIN]
    kB = -(-cnt[:, :, 1].max(axis=0) // 128)                  # [N_WIN]
    SA = int(kA.sum())
    SB = int(kB.sum())

    idx_lo = (src_s + 1).astype(np.int16)
    idx_hi = (src_s - SPLIT + 1).astype(np.int16)

    offA = np.zeros(N_WIN + 1, dtype=np.int64)
    np.cumsum(kA, out=offA[1:])
    offB = np.zeros(N_WIN + 1, dtype=np.int64)
    np.cumsum(kB, out=offB[1:])

    per_core = []
    for c in range(N_CORES):
        iA = np.zeros(SA * 128, dtype=np.int16)
        dA = np.zeros(SA * 128, dtype=np.float32)
        iB = np.zeros(max(SB, 1) * 128, dtype=np.int16)
        dB = np.zeros(max(SB, 1) * 128, dtype=np.float32)
        for w in range(N_WIN):
            g = (c * N_WIN + w) * 2
            base = (c * N_WIN + w) * WIN
            s0, s1 = starts[g], starts[g + 1]
            p0 = int(offA[w]) * 128
            iA[p0 : p0 + (s1 - s0)] = idx_lo[s0:s1]
            dA[p0 : p0 + (s1 - s0)] = (dst_s[s0:s1] - base).astype(np.float32)
            s0, s1 = starts[g + 1], starts[g + 2]
            p0 = int(offB[w]) * 128
            iB[p0 : p0 + (s1 - s0)] = idx_hi[s0:s1]
            dB[p0 : p0 + (s1 - s0)] = (dst_s[s0:s1] - base).astype(np.float32)
        per_core.append((iA, dA, iB, dB))

    return xlo, xhi, kA, kB, SA, SB, offA, offB, per_core


def _wrap_idx(idx_flat):
    """int16 stream -> dma_gather layout [128, n/16]: value i at
    [i % 16, i // 16], replicated across the 8 groups of 16 partitions."""
    a = idx_flat.reshape(-1, 16).T
    return np.tile(a, (8, 1)).copy()


def _wrap_dst(d_flat):
    """f32 stream -> [128, S]: subtile s lane e at [e, s]."""
    return np.ascontiguousarray(d_flat.reshape(-1, 128).T)


def _build_program(kA, kB, SA, SB, offA, offB):
    nc = bacc.Bacc(
        "TRN2", target_bir_lowering=False, debug=False, num_swdge_queues=NQ
    )

    t_xlo = nc.dram_tensor("xlo", [SPLIT + 1, ROW], F32, kind="ExternalInput")
    t_xhi = nc.dram_tensor(
        "xhi", [N_NODES - SPLIT + 1, ROW], F32, kind="ExternalInput"
    )
    t_wt = nc.dram_tensor("wt", [D, D], F32, kind="ExternalInput")
    t_b = nc.dram_tensor("bias", [D, 1], F32, kind="ExternalInput")
    t_ia = nc.dram_tensor("idxa", [128, SA * 8], I16, kind="ExternalInput")
    t_da = nc.dram_tensor("dsta", [128, SA], F32, kind="ExternalInput")
    SBp = max(SB, 1)
    t_ib = nc.dram_tensor("idxb", [128, SBp * 8], I16, kind="ExternalInput")
    t_db = nc.dram_tensor("dstb", [128, SBp], F32, kind="ExternalInput")
    t_iota = nc.dram_tensor("iota", [128, CHUNK * WIN], F32, kind="ExternalInput")
    t_id = nc.dram_tensor("ident", [D, D], F32, kind="ExternalInput")
    t_out = nc.dram_tensor("out", [D, NPC], F32, kind="ExternalOutput")

    callsA = [(p, min(CHUNK, SA - p)) for p in range(0, SA, CHUNK)]
    callsB = [(p, min(CHUNK, SB - p)) for p in range(0, SB, CHUNK)]

    with tile.TileContext(nc) as tc:
        with (
            tc.tile_pool(name="const", bufs=1) as cpool,
            tc.tile_pool(name="idx", bufs=1) as ipool,
            tc.tile_pool(name="msgsa", bufs=4) as mpa,
            tc.tile_pool(name="msgsb", bufs=3) as mpb,
            tc.tile_pool(name="oha", bufs=4) as opa,
            tc.tile_pool(name="ohb", bufs=3) as opb,
            tc.tile_pool(name="norm", bufs=4) as npool,
            tc.tile_pool(name="hpo", bufs=2) as hpool,
            tc.tile_pool(name="psacc", bufs=4, space="PSUM") as ps_acc,
            tc.tile_pool(name="pstr", bufs=2, space="PSUM") as ps_tr,
            tc.tile_pool(name="psz", bufs=2, space="PSUM") as ps_z,
        ):
            # ---- constants (iota / identity supplied from host) ----
            ident = cpool.tile([D, D], F32)
            nc.sync.dma_start(out=ident[:], in_=t_id[:])
            wt_sb = cpool.tile([D, D], F32)
            nc.sync.dma_start(out=wt_sb[:], in_=t_wt[:])
            b_sb = cpool.tile([D, 1], F32)
            nc.sync.dma_start(out=b_sb[:], in_=t_b[:])
            iota_f = cpool.tile([128, CHUNK * WIN], F32)
            nc.sync.dma_start(out=iota_f[:], in_=t_iota[:])

            ia_sb = ipool.tile([128, SA * 8], I16)
            nc.sync.dma_start(out=ia_sb[:], in_=t_ia[:])
            da_sb = ipool.tile([128, SA], F32)
            nc.sync.dma_start(out=da_sb[:], in_=t_da[:])
            ib_sb = ipool.tile([128, SBp * 8], I16)
            nc.sync.dma_start(out=ib_sb[:], in_=t_ib[:])
            db_sb = ipool.tile([128, SBp], F32)
            nc.sync.dma_start(out=db_sb[:], in_=t_db[:])

            out_sb = cpool.tile([D, NPC], F32)

            chunk_tiles = {0: [], 1: []}
            call_no = [0]

            def emit_chunk(st, k):
                if st == 0:
                    pos, nsub = callsA[k]
                    mp, op, tsrc, isb, dsb = mpa, opa, t_xlo, ia_sb, da_sb
                else:
                    pos, nsub = callsB[k]
                    mp, op, tsrc, isb, dsb = mpb, opb, t_xhi, ib_sb, db_sb
                msgs = mp.tile([128, CHUNK, ROW], F32)
                nidx = nsub * 128
                # single_packet=False: one packet per descriptor. The default
                # coalesces the whole call into one SDMA packet, which wedges
                # the engine beyond ~64 descriptors/lane (num_idxs >~ 1000).
                # Rotating queue_num spreads descriptor generation + ring
                # drain over the 4 SWDGE queues (~2x measured).
                nc.gpsimd.dma_gather(
                    msgs[:, :nsub, :],
                    tsrc[:],
                    isb[:, pos * 8 : pos * 8 + nsub * 8],
                    nidx,
                    nidx,
                    ROW,
                    single_packet=True,
                    queue_num=call_no[0] % NQ,
                )
                call_no[0] += 1
                oh = op.tile([128, CHUNK * WIN], F32)
                dst_b = (
                    dsb[:, pos : pos + nsub]
                    .unsqueeze(2)
                    .to_broadcast([128, nsub, WIN])
                )
                nc.vector.tensor_tensor(
                    out=oh[:, : nsub * WIN].rearrange("p (s w) -> p s w", w=WIN),
                    in0=iota_f[:, : nsub * WIN].rearrange(
                        "p (s w) -> p s w", w=WIN
                    ),
                    in1=dst_b,
                    op=mybir.AluOpType.is_equal,
                )
                chunk_tiles[st].append((msgs, oh))

            cursor = [0, 0]

            def tiles_for(st, s):
                k = s // CHUNK
                while cursor[st] <= k:
                    emit_chunk(st, cursor[st])
                    cursor[st] += 1
                msgs, oh = chunk_tiles[st][k]
                return msgs, oh, s % CHUNK

            pst = None
            for w in range(N_WIN):
                subs = [(0, int(offA[w]) + j) for j in range(int(kA[w]))]
                subs += [(1, int(offB[w]) + j) for j in range(int(kB[w]))]
                ps = ps_acc.tile([WIN, D + 1], F32)
                for j, (st, s) in enumerate(subs):
                    msgs, oh, col = tiles_for(st, s)
                    nc.tensor.matmul(
                        out=ps[:],
                        lhsT=oh[:, col * WIN : (col + 1) * WIN],
                        rhs=msgs[:, col, : D + 1],
                        start=(j == 0),
                        stop=(j == len(subs) - 1),
                    )
                deg = npool.tile([WIN, 1], F32)
                nc.vector.tensor_scalar_max(deg[:], ps[:, D : D + 1], 1.0)
                rec = npool.tile([WIN, 1], F32)
                nc.vector.reciprocal(rec[:], deg[:])
                h_w = npool.tile([WIN, D], F32)
                nc.vector.tensor_scalar_mul(h_w[:], ps[:, :D], rec[:])
                half = w % 2
                if half == 0:
                    pst = ps_tr.tile([D, 128], F32)
                nc.tensor.transpose(
                    out=pst[:, half * WIN : half * WIN + WIN],
                    in_=h_w[:],
                    identity=ident[:],
                )
                if half == 1:
                    ht = hpool.tile([D, 128], F32)
                    nc.vector.tensor_copy(out=ht[:], in_=pst[:])
                    z = ps_z.tile([D, 128], F32)
                    nc.tensor.matmul(
                        out=z[:], lhsT=wt_sb[:], rhs=ht[:], start=True, stop=True
                    )
                    t0 = (w // 2) * 128
                    nc.vector.tensor_scalar_add(
                        out_sb[:, t0 : t0 + 128], z[:], b_sb[:]
                    )

            nc.sync.dma_start(out=t_out[:], in_=out_sb[:])

    nc.compile()
    return nc


def kernel(x, src, dst, W, b):
    x = np.asarray(x, dtype=np.float32)
    W = np.asarray(W, dtype=np.float32)
    b = np.asarray(b, dtype=np.float32)

    xlo, xhi, kA, kB, SA, SB, offA, offB, per_core = _prep(x, src, dst)
    nc = _build_program(kA, kB, SA, SB, offA, offB)

    wt = np.ascontiguousarray(W.T)
    bcol = np.ascontiguousarray(b.reshape(D, 1))
    iota_arr = np.tile(
        np.arange(WIN, dtype=np.float32)[None, :], (128, CHUNK)
    ).copy()
    ident_arr = np.eye(D, dtype=np.float32)

    in_maps = []
    for c in range(N_CORES):
        iA, dA, iB, dB = per_core[c]
        in_maps.append(
            {
                "xlo": xlo,
                "xhi": xhi,
                "wt": wt,
                "bias": bcol,
                "idxa": _wrap_idx(iA),
                "dsta": _wrap_dst(dA),
                "idxb": _wrap_idx(iB),
                "dstb": _wrap_dst(dB),
                "iota": iota_arr,
                "ident": ident_arr,
            }
        )

    res = run_bass_kernel_spmd(nc, in_maps, list(range(N_CORES)))
    LAST["results"] = res
    LAST["exec_time_ns"] = res.exec_time_ns

    out_t = np.concatenate([res.results[c]["out"] for c in range(N_CORES)], axis=1)
    return np.ascontiguousarray(out_t.T[:N_NODES])

